# revision 1
# baseline (speedup 1.0000x reference)
"""Trainium2 Bass kernel for nn_AttentionModule_50002009260608.

B=16, C=512, H=W=24 (HW=576), TF=512, NH=8, CPH=64.
Data-parallel over batch: 2 batch elements per core x 8 cores.
Weights replicated; host pre-transposes 1x1-conv weights to [c_in, c_out]
and precomputes the two tiny text matvecs (t_m, Wm2 @ t).

All heavy matmuls run as float32r (full PE rate for N>=256) accumulating
in fp32 PSUM. fp32r ISA restrictions: output must span all 128 PE columns
(M>=97) and innermost AP counts must be even -- hence hw m-tiles of
116+115*4 and the padded per-head V'T stride of 128.
"""

import ml_dtypes
import numpy as np
from contextlib import ExitStack

import concourse.bacc as bacc
import concourse.bass as bass
import concourse.tile as tile
import concourse.mybir as mybir
from concourse import masks
from concourse.bass_utils import run_bass_kernel_spmd

B, C, HW, TF, NH, CPH = 16, 512, 576, 512, 8, 64
NCORES, BPC = 8, B // 8
SCALE = 1.0 / 8.0  # 1/sqrt(CPH)
F32, F32R = mybir.dt.float32, mybir.dt.float32r
BF16 = mybir.dt.bfloat16
AF = mybir.ActivationFunctionType
OP = mybir.AluOpType
PD = 128
NCC = C // PD                                    # 4 channel chunks
MT = [(0, 116), (116, 115), (231, 115), (346, 115), (461, 115)]  # hw m-tiles
NHALF = [(0, 288), (288, 288)]                   # softmax eviction halves
AVCH = [(0, 288), (288, 290)]                    # AV rhs chunks over es cols
CPS = 128                                        # padded per-head V'T stride
TMP = 104                                        # padded t_m_blk cols (fp32r M>=97)
ESW = HW + 2                                     # es cols: 576 + cross col + pad


def _body(ctx: ExitStack, tc, d):
    """d: DRAM APs: x[2,512,576](f32r), t_m_blk[2,512,104](f32r),
    tvec[2,512,1], WqT/WkT/WvT/Wm1T/WrT [512,512](f32r, pre-transposed
    [c_in,c_out]), Wr_b[512,1], out[2,512,576]."""
    nc = tc.nc

    wt = ctx.enter_context(tc.tile_pool(name="wt", bufs=1))
    act = ctx.enter_context(tc.tile_pool(name="act", bufs=1))
    expp = ctx.enter_context(tc.tile_pool(name="expp", bufs=1))
    ps = ctx.enter_context(tc.tile_pool(name="ps", bufs=1, space="PSUM"))

    # ---- batch-0 activations first (PE can start within ~2us), then weights,
    # all split per channel-chunk so the first conv group's deps arrive early ----
    xbts = []
    for b in range(BPC):
        xbt = act.tile([PD, NCC * HW], BF16, name=f"xb{b}", tag="xb", bufs=2)
        if b == 0:
            nc.sync.dma_start(xbt[:].rearrange("p (cc n) -> p cc n", cc=NCC),
                              d["x"][b].rearrange("(cc p) n -> p cc n", p=PD))
        xbts.append(xbt)
    W = {}
    for wn in ("WqT", "WkT", "Wm1T", "WvT", "WrT"):
        dt = F32R if wn == "WrT" else BF16
        wtile = wt.tile([PD, NCC * C], dt, name=f"{wn}_t")
        nc.scalar.dma_start(wtile[:].rearrange("p (cc o) -> p cc o", cc=NCC),
                            d[wn].rearrange("(cc p) o -> p cc o", p=PD))
        W[wn] = [wtile[:, j * C:(j + 1) * C] for j in range(NCC)]
    wrbt = wt.tile([PD, NCC], F32, name="wrbt")
    nc.sync.dma_start(wrbt[:], d["Wr_b"].rearrange("(cc p) one -> p (cc one)", p=PD))
    wrb = [wrbt[:, j:j + 1] for j in range(NCC)]
    ident = wt.tile([PD, PD], F32, name="ident")
    masks.make_identity(nc, ident[:])
    onesb = wt.tile([PD, (CPS - CPH) * NH], F32, name="onesb")
    nc.vector.memset(onesb[:], 1.0)
    # batched softmax-sum reciprocal scratch: a sub-head's 2 sums rows parked
    # at partitions 0/32, one [33, 288] reciprocal covers both halves.
    # Two tile sets, alternated by sub parity, so consecutive sub-heads pipeline.
    smt2 = [wt.tile([33, 288], F32, name=f"smt{i}") for i in range(2)]
    smr2 = [wt.tile([33, 288], F32, name=f"smr{i}") for i in range(2)]
    smb2 = [wt.tile([1, 288], F32, name=f"smb{i}") for i in range(2)]
    for i in range(2):
        nc.vector.memset(smt2[i][:], 1.0)
    # persistent V'T tiles: [hw_tile, 8*128]; per head block: cols 0:64 = V_h^T,
    # cols 64:128 = 1.0 (fused softmax column sums). Ones written once.
    VT = [wt.tile([sz, NH * CPS], BF16, name=f"vt{mi}")
          for mi, (m0, sz) in enumerate(MT)]
    for mi, (m0, sz) in enumerate(MT):
        nc.vector.tensor_copy(
            VT[mi][:].rearrange("p (h c) -> p h c", h=NH)[:, :, CPH:CPS],
            onesb[0:sz, :])

    def conv(name, b, Wn, rhs, outs, bias=None):
        # outs[ot][:, n] = sum_cc Wn[cc][:, ot*128:+128].T @ rhs[cc][:, n] (+ bias)
        for ot in range(NCC):
            for (n0, nsz) in NHALF:
                p = ps.tile([PD, nsz], F32, tag="conv", bufs=2,
                            name=f"p_{name}{b}_{ot}_{n0}")
                for cc in range(NCC):
                    nc.tensor.matmul(
                        p[:], Wn[cc][:, ot * PD:(ot + 1) * PD],
                        rhs[cc][:, n0:n0 + nsz],
                        start=(cc == 0), stop=(cc == NCC - 1))
                dst = outs[ot][:, n0:n0 + nsz]
                if bias is not None:
                    nc.scalar.activation(dst, p[:], AF.Identity, bias=bias[ot])
                else:
                    nc.vector.tensor_copy(dst, p[:])

    st8 = {}

    def emit_loads(b):
        xbt = xbts[b]
        if b > 0:
            nc.sync.dma_start(xbt[:].rearrange("p (cc n) -> p cc n", cc=NCC),
                              d["x"][b].rearrange("(cc p) n -> p cc n", p=PD))
        xb = [xbt[:, j * HW:(j + 1) * HW] for j in range(NCC)]
        tvt = act.tile([PD, NCC], F32, name=f"tv{b}", tag="tv")
        nc.sync.dma_start(tvt[:],
                          d["tvec"][b].rearrange("(cc p) one -> p (cc one)", p=PD))
        tmbt = act.tile([PD, NCC * TMP], BF16, name=f"tmblk{b}", tag="tmblk")
        nc.sync.dma_start(tmbt[:].rearrange("p (cc h) -> p cc h", cc=NCC),
                          d["t_m_blk"][b].rearrange("(cc p) h -> p cc h", p=PD))
        st8[b] = {
            "xb": xb,
            "tvecs": [tvt[:, j:j + 1] for j in range(NCC)],
            "tmblk": [tmbt[:, j * TMP:(j + 1) * TMP] for j in range(NCC)],
        }

    def emit_q(b):
        s = st8[b]
        s["Q"] = [act.tile([PD, HW], BF16, name=f"q{b}_{j}", tag=f"q{j}", bufs=2)
                  for j in range(NCC)]
        conv("q", b, W["WqT"], s["xb"], s["Q"])

    def emit_k(b):
        s = st8[b]
        s["K"] = [act.tile([PD, HW], BF16, name=f"k{b}_{j}", tag=f"k{j}", bufs=2)
                  for j in range(NCC)]
        conv("k", b, W["WkT"], s["xb"], s["K"])

    def emit_vl_vt(b):
        s = st8[b]
        vl = [act.tile([PD, HW], BF16, name=f"vl{b}_{j}", tag=f"vl{j}")
              for j in range(NCC)]
        conv("vl", b, W["Wm1T"], s["xb"], vl, bias=s["tvecs"])
        for mi, (m0, sz) in enumerate(MT):
            p = ps.tile([sz, C], F32, tag="conv", bufs=2, name=f"p_vt{b}_{mi}")
            for cc in range(NCC):
                nc.tensor.matmul(p[:], vl[cc][:, m0:m0 + sz], W["WvT"][cc][:],
                                 start=(cc == 0), stop=(cc == NCC - 1))
            vsrc = p[:].rearrange("p (h c) -> p h c", h=NH)
            vv = VT[mi][:].rearrange("p (h c) -> p h c", h=NH)
            nc.vector.tensor_copy(vv[:, :, 0:CPH], vsrc)

    def emit_cross(b):
        s = st8[b]
        xb, tmblk = s["xb"], s["tmblk"]
        crosse = act.tile([NH, HW], F32, name=f"crosse{b}", tag="crosse")
        csum = [act.tile([NH, 1], F32, name=f"csum{b}_{i}", tag=f"csum{i}")
                for i in range(2)]
        for hi, (n0, nsz) in enumerate(NHALF):
            p = ps.tile([TMP, nsz], F32, tag="s", bufs=3, name=f"p_cl{b}_{hi}")
            for cc in range(NCC):
                nc.tensor.matmul(p[:], tmblk[cc], xb[cc][:, n0:n0 + nsz],
                                 start=(cc == 0), stop=(cc == NCC - 1))
            nc.scalar.activation(crosse[:, n0:n0 + nsz], p[0:NH, :], AF.Exp,
                                 scale=SCALE, accum_out=csum[hi][:])
        crec = act.tile([NH, 1], F32, name=f"crec{b}", tag="crec")
        nc.vector.tensor_add(crec[:], csum[0][:], csum[1][:])
        nc.vector.reciprocal(crec[:], crec[:])
        crossn = act.tile([NH, HW], F32, name=f"crossn{b}", tag="crossn")
        nc.vector.tensor_scalar_mul(crossn[:], crosse[:], crec[:])
        crossT = [act.tile([sz, NH + 1], BF16, name=f"crossT{b}_{mi}",
                           tag=f"crossT{mi}") for mi, (m0, sz) in enumerate(MT)]
        for mi, (m0, sz) in enumerate(MT):
            pt = ps.tile([sz, NH], F32, tag="conv", bufs=2, name=f"p_ct{b}_{mi}")
            nc.tensor.transpose(pt[:], crossn[:, m0:m0 + sz], ident[0:NH, 0:NH])
            nc.vector.tensor_copy(crossT[mi][0:sz, 0:NH], pt[:])
            nc.gpsimd.tensor_copy(crossT[mi][0:sz, NH:NH + 1], onesb[0:sz, 0:1])
        s["crossT"] = crossT
        s["outall"] = [act.tile([PD, HW], F32R, name=f"oa{b}_{j}", tag=f"oa{j}")
                       for j in range(NCC)]

    def emit_pair(b, hp):
        s = st8[b]
        K, Q, crossT, outall = s["K"], s["Q"], s["crossT"], s["outall"]
        h2 = (2 * hp, 2 * hp + 1)
        es = [[expp.tile([sz, ESW], BF16, name=f"es{b}_{hp}_{sub}_{mi}",
                         tag=f"es{sub}_{mi}", bufs=3)
               for mi, (m0, sz) in enumerate(MT)] for sub in range(2)]
        for mi, (m0, sz) in enumerate(MT):
            for hi, (n0, nsz) in enumerate(NHALF):
                for sub in range(2):
                    rr = sub * CPH
                    p = ps.tile([sz, nsz], F32, tag="s", bufs=3,
                                name=f"p_s{b}_{hp}_{sub}_{mi}_{n0}")
                    nc.tensor.matmul(
                        p[:], K[hp][rr:rr + CPH, m0:m0 + sz],
                        Q[hp][rr:rr + CPH, n0:n0 + nsz],
                        start=True, stop=True, tile_position=(rr, 0),
                        skip_group_check=True)
                    nc.scalar.activation(es[sub][mi][:, n0:n0 + nsz], p[:],
                                         AF.Exp, scale=SCALE)
            for sub in range(2):
                nc.gpsimd.tensor_copy(
                    es[sub][mi][:, HW:ESW],
                    crossT[mi][0:sz, h2[sub]:h2[sub] + 2])
        for sub in range(2):
            h = h2[sub]
            pav = [ps.tile([PD, nsz], F32, tag="av", bufs=3,
                           name=f"p_av{b}_{h}_{ci}")
                   for ci, (c0, nsz) in enumerate(AVCH)]
            for mi, (m0, sz) in enumerate(MT):
                lhs = VT[mi][:, h * CPS:(h + 1) * CPS]
                st, sp = (mi == 0), (mi == len(MT) - 1)
                for ci, (c0, nsz) in enumerate(AVCH):
                    nc.tensor.matmul(pav[ci][:], lhs,
                                     es[sub][mi][:, c0:c0 + nsz],
                                     start=st, stop=sp)
            rr = sub * CPH
            smt = smt2[(2 * hp + sub) % 2]
            smr = smr2[(2 * hp + sub) % 2]
            smb = smb2[(2 * hp + sub) % 2]
            nc.vector.tensor_copy(smt[0:1, :], pav[0][CPH:CPH + 1, 0:288])
            nc.vector.tensor_copy(smt[32:33, :], pav[1][CPH:CPH + 1, 0:288])
            nc.vector.reciprocal(smr[:], smt[:])
            nc.vector.tensor_copy(smb[:], smr[32:33, :])
            rep = act.tile([CPH, HW], F32, name=f"rep{b}_{2 * hp + sub}",
                           tag="rep", bufs=2)
            nc.gpsimd.partition_broadcast(rep[:, 0:288], smr[0:1, :])
            nc.gpsimd.partition_broadcast(rep[:, 288:HW], smb[:])
            dst = outall[hp][rr:rr + CPH, :]
            nc.vector.tensor_tensor(dst[:, 0:288], pav[0][0:CPH, :],
                                    rep[:, 0:288], OP.mult)
            nc.vector.tensor_tensor(dst[:, 288:HW], pav[1][0:CPH, 0:288],
                                    rep[:, 288:HW], OP.mult)
            nc.vector.tensor_scalar_add(dst, dst, pav[1][0:CPH, 288:289])

    def emit_final(b, ots=range(NCC)):
        s = st8[b]
        if "fin" not in s:
            s["fin"] = [act.tile([PD, HW], F32, name=f"fin{b}_{j}", tag=f"fin{j}")
                        for j in range(NCC)]
        fin = s["fin"]
        for ot in ots:
            for (n0, nsz) in NHALF:
                p = ps.tile([PD, nsz], F32, tag="conv", bufs=2,
                            name=f"p_fin{b}_{ot}_{n0}")
                for cc in range(NCC):
                    nc.tensor.matmul(
                        p[:], W["WrT"][cc][:, ot * PD:(ot + 1) * PD],
                        s["outall"][cc][:, n0:n0 + nsz],
                        start=(cc == 0), stop=(cc == NCC - 1))
                nc.scalar.activation(fin[ot][:, n0:n0 + nsz], p[:], AF.Identity,
                                     bias=wrb[ot])
            nc.sync.dma_start(d["out"][b, ot * PD:(ot + 1) * PD, :], fin[ot][:])

    # interleave batch 1's PE-dense conv work into batch 0's head phase so the
    # tensor engine stays busy (and the HAM clock stays warm) throughout.
    emit_loads(0)
    emit_q(0)
    emit_k(0)
    emit_vl_vt(0)
    emit_cross(0)
    emit_loads(1)
    emit_pair(0, 0)
    emit_q(1)
    emit_pair(0, 1)
    emit_k(1)
    emit_pair(0, 2)
    emit_pair(0, 3)
    emit_vl_vt(1)
    emit_cross(1)
    emit_pair(1, 0)
    emit_final(0, [0])
    emit_pair(1, 1)
    emit_final(0, [1])
    emit_pair(1, 2)
    emit_final(0, [2])
    emit_pair(1, 3)
    emit_final(0, [3])
    emit_final(1)


_CACHE = {}


def _build():
    if "nc" in _CACHE:
        return _CACHE["nc"], _CACHE["out"]
    nc = bacc.Bacc("TRN2", target_bir_lowering=False, debug=False,
                   num_devices=NCORES)
    d = {
        "x": nc.dram_tensor("x", [BPC, C, HW], BF16, kind="ExternalInput").ap(),
        "t_m_blk": nc.dram_tensor("t_m_blk", [BPC, C, TMP], BF16,
                                  kind="ExternalInput").ap(),
        "tvec": nc.dram_tensor("tvec", [BPC, C, 1], F32, kind="ExternalInput").ap(),
        "Wr_b": nc.dram_tensor("Wr_b", [C, 1], F32, kind="ExternalInput").ap(),
        "out": nc.dram_tensor("out", [BPC, C, HW], F32, kind="ExternalOutput").ap(),
    }
    for wn in ("WqT", "WkT", "WvT", "Wm1T"):
        d[wn] = nc.dram_tensor(wn, [C, C], BF16, kind="ExternalInput").ap()
    d["WrT"] = nc.dram_tensor("WrT", [C, C], F32R, kind="ExternalInput").ap()
    with tile.TileContext(nc) as tc:
        with ExitStack() as ctx:
            _body(ctx, tc, d)
    nc.compile()
    _CACHE["nc"], _CACHE["out"] = nc, d["out"].tensor.name
    return nc, _CACHE["out"]


def _prep_inputs(x, t, Wk, Wq, Wt_w, Wt_b, Wm, Wv, Wr_w, Wr_b):
    f = np.float32
    x = np.asarray(x, f).reshape(B, C, HW)
    t = np.asarray(t, f)
    t_m = t @ np.asarray(Wt_w, f).T + np.asarray(Wt_b, f)
    t_m_blk = np.zeros((B, C, TMP), f)
    for h in range(NH):
        t_m_blk[:, h * CPH:(h + 1) * CPH, h] = t_m[:, h * CPH:(h + 1) * CPH]
    tvec = (t @ np.asarray(Wm, f)[:, C:].T).reshape(B, C, 1)
    bf = ml_dtypes.bfloat16
    com = {
        "WqT": np.ascontiguousarray(np.asarray(Wq, f).T).astype(bf),
        "WkT": np.ascontiguousarray(np.asarray(Wk, f).T).astype(bf),
        "WvT": np.ascontiguousarray(np.asarray(Wv, f).T).astype(bf),
        "Wm1T": np.ascontiguousarray(np.asarray(Wm, f)[:, :C].T).astype(bf),
        "WrT": np.ascontiguousarray(np.asarray(Wr_w, f).T),
        "Wr_b": np.asarray(Wr_b, f).reshape(C, 1),
    }
    maps = []
    for c in range(NCORES):
        sl = slice(c * BPC, (c + 1) * BPC)
        m = dict(com)
        m["x"] = np.ascontiguousarray(x[sl]).astype(bf)
        m["t_m_blk"] = np.ascontiguousarray(t_m_blk[sl]).astype(bf)
        m["tvec"] = np.ascontiguousarray(tvec[sl])
        maps.append(m)
    return maps


def kernel(x, t, Wk, Wq, Wt_w, Wt_b, Wm, Wv, Wr_w, Wr_b, _trace=False):
    nc, out_name = _build()
    maps = _prep_inputs(x, t, Wk, Wq, Wt_w, Wt_b, Wm, Wv, Wr_w, Wr_b)
    res = run_bass_kernel_spmd(nc, maps, core_ids=list(range(NCORES)),
                               trace=_trace)
    out = np.concatenate([res.results[c][out_name] for c in range(NCORES)],
                         axis=0).reshape(B, C, 24, 24)
    if _trace:
        kernel.last_results = res
    return out



# revision 6
# speedup vs baseline: 1.3185x; 1.3185x over previous
"""Trainium2 Bass kernel for nn_AttentionModule_50002009260608.

B=16, C=512, H=W=24 (HW=576), TF=512, NH=8, CPH=64.
Data-parallel over batch: 2 batch elements per core x 8 cores.

All matmuls bf16 (fp32 PSUM). Structure:
- QK psum tiles are [sz, 576] spanning 2 PSUM banks (writes split 512/64)
  so softmax exp is ONE wide ACT instruction per (sub-head, hw_k tile).
- V'T tiles carry a leading 64-wide ones block per head, so the AV matmul
  emits the softmax denominator replicated across psum partitions 0:64;
  1/D comes from reciprocal_approx_fast straight off PSUM (base-0
  partitions required by custom DVE ops) -- no partition_broadcast.
- AV accumulates per q-half into 1-bank [128, 288/289] psum tiles
  (av bufs=2) so the DVE normalize chain pipelines across halves.
- The additive cross-attention term rides as es column 576; its V@cross
  product flows through the qh1 normalize (its denominator is sum(cross)
  = 1.0, so the reciprocal multiply is a no-op) into outall col 576,
  which the final 1x1 conv turns into a per-partition bias (rank-1).
- Host pre-packs every DRAM tensor to contiguous [128, *] rows; weight
  DMAs split per channel-chunk so the first conv starts early.
- PE warmup matmuls ramp the p-state clock during the DMA prologue; a
  queue of fine-grained conv/final filler pieces for the other batch is
  drained between QK/AV groups to keep the PE gapless (and at full
  clock) while ACT/DVE chains drain.
- PSUM banks: s 2x2 + av 2x1 + conv 1x2 = 8.
"""

import ml_dtypes
import numpy as np
from collections import deque
from contextlib import ExitStack

import concourse.bacc as bacc
import concourse.bass as bass
import concourse.tile as tile
import concourse.mybir as mybir
from concourse import masks
from concourse.bass_utils import run_bass_kernel_spmd

B, C, HW, TF, NH, CPH = 16, 512, 24 * 24, 512, 8, 64
NCORES, BPC = 8, B // 8
SCALE = 1.0 / 8.0  # 1/sqrt(CPH)
F32 = mybir.dt.float32
BF16 = mybir.dt.bfloat16
AF = mybir.ActivationFunctionType
OP = mybir.AluOpType
PD = 128
NCC = C // PD                                    # 4 channel chunks
MT = [(0, 116), (116, 115), (231, 115), (346, 115), (461, 115)]  # hw_k tiles
NS = [(0, 512), (512, 64)]                       # big/small col splits
QH = [(0, 288), (288, 289)]                      # AV q-half splits (+cross)
CPS = 128                                        # per-head V'T stride
ESW = HW + 1                                     # es cols: 576 + cross col


def _body(ctx: ExitStack, tc, d):
    """DRAM (per core): x[2,128,4*576] bf16, tvec[2,128,4] f32,
    tmblk[2,128,4*8] bf16, WqT/WkT/WvT/Wm1T/WrT [128,4*512] bf16
    (host pre-transposed/packed), wrb[128,4] f32, out[2,512,576] f32."""
    nc = tc.nc

    wt = ctx.enter_context(tc.tile_pool(name="wt", bufs=1))
    act = ctx.enter_context(tc.tile_pool(name="act", bufs=1))
    expp = ctx.enter_context(tc.tile_pool(name="expp", bufs=1))
    ps = ctx.enter_context(tc.tile_pool(name="ps", bufs=1, space="PSUM"))

    # ---- dep-free setup first ----
    warm = wt.tile([PD, 640], BF16, name="warm")
    nc.vector.memset(warm[:], 0.125)
    ident = wt.tile([16, 16], F32, name="ident")
    masks.make_identity(nc, ident[:])

    xbts, st8 = [], {}
    for b in range(BPC):
        xbts.append(act.tile([PD, NCC * HW], BF16, name=f"xb{b}", tag="xb",
                             bufs=2))
        st8[b] = {}

    def emit_loads(b):
        s = st8[b]
        xbt = xbts[b]
        for cc in range(NCC):
            nc.sync.dma_start(xbt[:, cc * HW:(cc + 1) * HW],
                              d["x"][b][:, cc * HW:(cc + 1) * HW])
        tvt = act.tile([PD, NCC], F32, name=f"tv{b}", tag="tv", bufs=2)
        nc.sync.dma_start(tvt[:], d["tvec"][b])
        tmbt = act.tile([PD, NCC * NH], BF16, name=f"tmblk{b}", tag="tmblk",
                        bufs=2)
        nc.sync.dma_start(tmbt[:], d["tmblk"][b])
        s["xb"] = [xbt[:, j * HW:(j + 1) * HW] for j in range(NCC)]
        s["tvecs"] = [tvt[:, j:j + 1] for j in range(NCC)]
        s["tmblk"] = [tmbt[:, j * NH:(j + 1) * NH] for j in range(NCC)]
        # VT[mi]: [sz, 8*128]; per head block: cols 0:64 = ones (softmax
        # sums), 64:128 = V_h^T. Memset early (gpsimd is idle here).
        s["VT"] = [act.tile([sz, NH * CPS], BF16, name=f"vt{b}_{mi}",
                            tag=f"vt{mi}", bufs=2)
                   for mi, (m0, sz) in enumerate(MT)]
        for mi in range(len(MT)):
            nc.gpsimd.memset(s["VT"][mi][:], 1.0)

    emit_loads(0)
    W = {}
    for wn, eng in (("WqT", nc.scalar), ("WkT", nc.scalar),
                    ("Wm1T", nc.gpsimd), ("WvT", nc.gpsimd),
                    ("WrT", nc.gpsimd)):
        wtile = wt.tile([PD, NCC * C], BF16, name=f"{wn}_t")
        for cc in range(NCC):
            eng.dma_start(wtile[:, cc * C:(cc + 1) * C],
                          d[wn][:, cc * C:(cc + 1) * C])
        W[wn] = [wtile[:, j * C:(j + 1) * C] for j in range(NCC)]
    wrbt = wt.tile([PD, NCC], F32, name="wrbt")
    nc.gpsimd.dma_start(wrbt[:], d["wrb"])
    wrb = [wrbt[:, j:j + 1] for j in range(NCC)]

    # ---- PE warmup: ramp the tensor-engine clock while DMAs land ----
    for i in range(7):
        p = ps.tile([PD, 512], F32, tag="conv", bufs=1, name=f"pwarm{i}")
        nc.tensor.matmul(p[:], warm[:, 0:128], warm[:, 128:640],
                         start=True, stop=True)

    # ---- conv helpers; each (ot) split into big/small pieces so the
    # filler queue can interleave them between QK/AV groups ----
    def conv_big(name, b, Wn, rhs, ot, w=HW):
        p = ps.tile([PD, ESW], F32, tag="conv", bufs=1,
                    name=f"p_{name}{b}_{ot}")
        for cc in range(NCC):
            nc.tensor.matmul(p[:, 0:512], Wn[cc][:, ot * PD:(ot + 1) * PD],
                             rhs[cc][:, 0:512],
                             start=(cc == 0), stop=(cc == NCC - 1),
                             skip_group_check=True)
        return p

    def conv_small(p, name, b, Wn, rhs, ot, out, bias, w):
        for cc in range(NCC):
            nc.tensor.matmul(p[:, 512:w], Wn[cc][:, ot * PD:(ot + 1) * PD],
                             rhs[cc][:, 512:w],
                             start=(cc == 0), stop=(cc == NCC - 1),
                             skip_group_check=True)
        if name == "fin":
            # fin = psum + (Wr@crossvec)[per-partition] + Wr_b
            nc.vector.tensor_scalar(out[:], p[:, 0:HW], p[:, HW:ESW],
                                    bias, OP.add, OP.add)
            nc.sync.dma_start(d["out"][b, ot * PD:(ot + 1) * PD, :], out[:])
        elif bias is not None:
            nc.vector.tensor_scalar(out[:], p[:, 0:HW], bias, None, OP.add)
        else:
            nc.vector.tensor_copy(out[:], p[:, 0:HW])

    def conv_pieces(name, b, Wn, rhs_key, outs_key, bias_key=None, w=HW):
        # returns a list of filler closures (2 per ot, sharing one psum)
        pieces = []
        for ot in range(NCC):
            box = {}

            def big(ot=ot, box=box):
                s = st8[b]
                box["p"] = conv_big(name, b, Wn, s[rhs_key], ot, w)

            def small(ot=ot, box=box):
                s = st8[b]
                bias = s[bias_key][ot] if bias_key else (
                    wrb[ot] if name == "fin" else None)
                conv_small(box["p"], name, b, Wn, s[rhs_key], ot,
                           s[outs_key][ot], bias, w)
            pieces += [big, small]
        return pieces

    def alloc_qkvl(b):
        s = st8[b]
        for key, tg in (("Q", "q"), ("K", "k"), ("vl", "vl")):
            if key not in s:
                s[key] = [act.tile([PD, HW], BF16, name=f"{tg}{b}_{j}",
                                   tag=f"{tg}{j}", bufs=2)
                          for j in range(NCC)]

    def vt_pieces(b):
        s = st8[b]

        def mk(mi):
            def piece():
                m0, sz = MT[mi]
                p = ps.tile([sz, C], F32, tag="conv", bufs=1,
                            name=f"p_vt{b}_{mi}")
                for cc in range(NCC):
                    nc.tensor.matmul(p[:], s["vl"][cc][:, m0:m0 + sz],
                                     W["WvT"][cc][:],
                                     start=(cc == 0), stop=(cc == NCC - 1))
                vsrc = p[:].rearrange("p (h c) -> p h c", h=NH)
                vv = s["VT"][mi][:].rearrange("p (h c) -> p h c", h=NH)
                nc.vector.tensor_copy(vv[:, :, CPH:CPS], vsrc)
            return piece
        return [mk(mi) for mi in range(len(MT))]

    def emit_cross(b):
        s = st8[b]
        xb, tmblk = s["xb"], s["tmblk"]
        p = ps.tile([NH, HW], F32, tag="s", bufs=2, name=f"p_cl{b}")
        for (n0, nsz) in NS:
            for cc in range(NCC):
                nc.tensor.matmul(p[0:NH, n0:n0 + nsz], tmblk[cc],
                                 xb[cc][:, n0:n0 + nsz],
                                 start=(cc == 0), stop=(cc == NCC - 1),
                                 skip_group_check=True)
        crosse = act.tile([NH, HW], F32, name=f"crosse{b}", tag="crosse")
        csum = act.tile([NH, 1], F32, name=f"csum{b}", tag="csum")
        nc.scalar.activation(crosse[:], p[0:NH, :], AF.Exp, scale=SCALE,
                             accum_out=csum[:])
        crec = act.tile([NH, 1], F32, name=f"crec{b}", tag="crec")
        nc.vector.reciprocal(crec[:], csum[:])
        crossn = act.tile([NH, HW], F32, name=f"crossn{b}", tag="crossn")
        nc.vector.tensor_scalar_mul(crossn[:], crosse[:], crec[:])
        crossT = [act.tile([sz, NH], BF16, name=f"crossT{b}_{mi}",
                           tag=f"crossT{mi}", bufs=2)
                  for mi, (m0, sz) in enumerate(MT)]
        for mi, (m0, sz) in enumerate(MT):
            pt = ps.tile([sz, NH], F32, tag="s", bufs=2, name=f"p_ct{b}_{mi}")
            nc.tensor.transpose(pt[:], crossn[:, m0:m0 + sz],
                                ident[0:NH, 0:NH])
            nc.vector.tensor_copy(crossT[mi][:], pt[:])
        s["crossT"] = crossT
        s["outall"] = [act.tile([PD, ESW], BF16, name=f"oa{b}_{j}",
                                tag=f"oa{j}", bufs=2) for j in range(NCC)]

    fills = deque()

    def fill(n=1):
        for _ in range(n):
            if fills:
                fills.popleft()()

    def emit_pair(b, hp):
        s = st8[b]
        K, Q, crossT, outall, VT = (s["K"], s["Q"], s["crossT"], s["outall"],
                                    s["VT"])
        h2 = (2 * hp, 2 * hp + 1)
        es = [[expp.tile([sz, ESW], BF16, name=f"es{b}_{hp}_{sub}_{mi}",
                         tag=f"es{sub}_{mi}", bufs=2)
               for mi, (m0, sz) in enumerate(MT)] for sub in range(2)]
        for mi, (m0, sz) in enumerate(MT):
            for sub in range(2):
                rr = sub * CPH
                p = ps.tile([sz, HW], F32, tag="s", bufs=2,
                            name=f"p_s{b}_{hp}_{sub}_{mi}")
                for (n0, nsz) in NS:
                    nc.tensor.matmul(
                        p[:, n0:n0 + nsz], K[hp][rr:rr + CPH, m0:m0 + sz],
                        Q[hp][rr:rr + CPH, n0:n0 + nsz],
                        start=True, stop=True, tile_position=(rr, 0),
                        skip_group_check=True)
                nc.scalar.activation(es[sub][mi][:, 0:HW], p[:], AF.Exp,
                                     scale=SCALE)
                nc.gpsimd.tensor_copy(
                    es[sub][mi][:, HW:ESW],
                    crossT[mi][:, h2[sub]:h2[sub] + 1])
            fill()
        for sub in range(2):
            h = h2[sub]
            rr = sub * CPH
            for qh, (q0, qw) in enumerate(QH):
                pav = ps.tile([PD, qw], F32, tag="av", bufs=2,
                              name=f"p_av{b}_{h}_{qh}")
                for mi, (m0, sz) in enumerate(MT):
                    nc.tensor.matmul(pav[:], VT[mi][:, h * CPS:(h + 1) * CPS],
                                     es[sub][mi][:, q0:q0 + qw],
                                     start=(mi == 0), stop=(mi == len(MT) - 1))
                # psum rows 0:64 = D(q) replicated (ones block); 64:128 = AV.
                # For qh1 the last col is V@cross whose "denominator" is
                # sum(crossn) = 1.0, so the same normalize works.
                rep = act.tile([CPH, qw], F32, name=f"rep{b}_{h}_{qh}",
                               tag=f"rep{qh}", bufs=2)
                nc.vector.reciprocal_approx_fast(rep[:], pav[0:CPH, :])
                nc.vector.tensor_tensor(outall[hp][rr:rr + CPH, q0:q0 + qw],
                                        pav[CPH:PD, :], rep[:], OP.mult)
                fill()

    # ---- schedule ----
    alloc_qkvl(0)
    for piece in conv_pieces("q", 0, W["WqT"], "xb", "Q"):
        piece()
    for piece in conv_pieces("k", 0, W["WkT"], "xb", "K"):
        piece()
    emit_loads(1)
    for piece in conv_pieces("vl", 0, W["Wm1T"], "xb", "vl",
                             bias_key="tvecs"):
        piece()
    for piece in vt_pieces(0):
        piece()
    emit_cross(0)
    alloc_qkvl(1)
    fills.extend(conv_pieces("q", 1, W["WqT"], "xb", "Q"))
    fills.extend(conv_pieces("k", 1, W["WkT"], "xb", "K"))
    fills.extend(conv_pieces("vl", 1, W["Wm1T"], "xb", "vl",
                             bias_key="tvecs"))
    fills.extend(vt_pieces(1))
    for hp in range(4):
        emit_pair(0, hp)
    while fills:
        fill()
    emit_cross(1)
    st8[0]["fin"] = [act.tile([PD, HW], F32, name=f"fin0_{j}", tag=f"fin{j}",
                              bufs=2) for j in range(NCC)]
    fills.extend(conv_pieces("fin", 0, W["WrT"], "outall", "fin", w=ESW))
    for hp in range(4):
        emit_pair(1, hp)
    while fills:
        fill()
    st8[1]["fin"] = [act.tile([PD, HW], F32, name=f"fin1_{j}", tag=f"fin{j}",
                              bufs=2) for j in range(NCC)]
    for piece in conv_pieces("fin", 1, W["WrT"], "outall", "fin", w=ESW):
        piece()


_CACHE = {}


def _build():
    if "nc" in _CACHE:
        return _CACHE["nc"], _CACHE["out"]
    nc = bacc.Bacc("TRN2", target_bir_lowering=False, debug=False,
                   num_devices=NCORES)
    d = {
        "x": nc.dram_tensor("x", [BPC, PD, NCC * HW], BF16,
                            kind="ExternalInput").ap(),
        "tvec": nc.dram_tensor("tvec", [BPC, PD, NCC], F32,
                               kind="ExternalInput").ap(),
        "tmblk": nc.dram_tensor("tmblk", [BPC, PD, NCC * NH], BF16,
                                kind="ExternalInput").ap(),
        "wrb": nc.dram_tensor("wrb", [PD, NCC], F32,
                              kind="ExternalInput").ap(),
        "out": nc.dram_tensor("out", [BPC, C, HW], F32,
                              kind="ExternalOutput").ap(),
    }
    for wn in ("WqT", "WkT", "WvT", "Wm1T", "WrT"):
        d[wn] = nc.dram_tensor(wn, [PD, NCC * C], BF16,
                               kind="ExternalInput").ap()
    with tile.TileContext(nc) as tc:
        with ExitStack() as ctx:
            _body(ctx, tc, d)
    nc.compile()
    _CACHE["nc"], _CACHE["out"] = nc, d["out"].tensor.name
    return nc, _CACHE["out"]


def _packc(M):
    # [C, X] -> [128, 4*X]: partition p, chunk cc <- row cc*128+p
    X = M.shape[1]
    return np.ascontiguousarray(
        M.reshape(NCC, PD, X).transpose(1, 0, 2).reshape(PD, NCC * X))


def _prep_inputs(x, t, Wk, Wq, Wt_w, Wt_b, Wm, Wv, Wr_w, Wr_b):
    f = np.float32
    bf = ml_dtypes.bfloat16
    x = np.asarray(x, f).reshape(B, C, HW)
    t = np.asarray(t, f)
    t_m = t @ np.asarray(Wt_w, f).T + np.asarray(Wt_b, f)       # [B, C]
    tmblk = np.zeros((B, C, NH), f)
    for h in range(NH):
        tmblk[:, h * CPH:(h + 1) * CPH, h] = t_m[:, h * CPH:(h + 1) * CPH]
    tvec = t @ np.asarray(Wm, f)[:, C:].T                        # [B, C]
    com = {
        "WqT": _packc(np.asarray(Wq, f).T).astype(bf),
        "WkT": _packc(np.asarray(Wk, f).T).astype(bf),
        "WvT": _packc(np.asarray(Wv, f).T).astype(bf),
        "Wm1T": _packc(np.asarray(Wm, f)[:, :C].T).astype(bf),
        "WrT": _packc(np.asarray(Wr_w, f).T).astype(bf),
        "wrb": np.ascontiguousarray(np.asarray(Wr_b, f).reshape(NCC, PD).T),
    }
    xp = x.reshape(B, NCC, PD, HW).transpose(0, 2, 1, 3).reshape(
        B, PD, NCC * HW).astype(bf)
    tvp = np.ascontiguousarray(tvec.reshape(B, NCC, PD).transpose(0, 2, 1))
    tmp_ = tmblk.reshape(B, NCC, PD, NH).transpose(0, 2, 1, 3).reshape(
        B, PD, NCC * NH).astype(bf)
    maps = []
    for c in range(NCORES):
        sl = slice(c * BPC, (c + 1) * BPC)
        m = dict(com)
        m["x"] = np.ascontiguousarray(xp[sl])
        m["tvec"] = np.ascontiguousarray(tvp[sl])
        m["tmblk"] = np.ascontiguousarray(tmp_[sl])
        maps.append(m)
    return maps


def kernel(x, t, Wk, Wq, Wt_w, Wt_b, Wm, Wv, Wr_w, Wr_b, _trace=False):
    nc, out_name = _build()
    maps = _prep_inputs(x, t, Wk, Wq, Wt_w, Wt_b, Wm, Wv, Wr_w, Wr_b)
    res = run_bass_kernel_spmd(nc, maps, core_ids=list(range(NCORES)),
                               trace=_trace)
    out = np.concatenate([res.results[c][out_name] for c in range(NCORES)],
                         axis=0).reshape(B, C, 24, 24)
    if _trace:
        kernel.last_results = res
    return out
